# revision 14
# baseline (speedup 1.0000x reference)
"""Trainium2 Bass kernel for a pre-LN transformer block (CLIP-style, QuickGELU)
applied to two ragged groups x1:[8,1024,D], x2:[16,256,D], D=1024, H=16.

Sharding: data-parallel over sequences. Core i handles x1[i] (1024 tokens) +
x2[2i], x2[2i+1] (256 tokens each) = 1536 tokens. Weights replicated.

On-chip layout strategy ("feature-major spine"):
  - LN stats/apply in token-major (tokens on partitions), then PE-transpose
    x_ln into d-major [d, t] for all contraction-over-d matmuls.
  - Q,K produced feature-major [dh, t]; V token-major [t, dh] (swapped matmul
    args), augmented with a ones-column so the attn@V matmul also produces the
    softmax denominator (row 64 of PSUM).
  - Scores computed transposed: S_T[k,q] via lhsT=K_h[dh,ktile],
    rhs=Q_h[dh,q]; softmax without max-subtraction (scores are small here);
    one fused exp per head-pair straight out of PSUM on ScalarE.
  - All matmuls run in float32r (TF32-like; ~250ns per 128x128x512 on HW).
"""

import sys

sys.path.insert(0, "/opt/trn_rl_repo")

from contextlib import ExitStack

import numpy as np

import concourse.bass as bass
import concourse.tile as tile
from concourse import bacc, mybir
from concourse.bass_utils import run_bass_kernel_spmd
from concourse.masks import make_identity

P = 128
D = 1024
H = 16
DH = 64
D4 = 4096
T = 1536  # tokens per core
TT = T // P  # 12 token tiles
KO = D // P  # 8
KO4 = D4 // P  # 32
EPS = 1e-5
SEQS = ((0, 1024), (1024, 256), (1280, 256))  # (t0, n) per core
TG = 768  # MLP token-group size
NTG = T // TG
N_CORES = 8
F32 = mybir.dt.float32
F32R = mybir.dt.float32r
AF = mybir.ActivationFunctionType
OP = mybir.AluOpType

_cache = {}


def _ln(nc, pools, x_t, xln_t, g_b, be_b):
    """LayerNorm token-major: x_t [P, D] fp32 -> xln_t [P, D] fp32.
    Stats on DVE, apply on ACT. g_b/be_b optional [P, D] broadcast tiles."""
    st = pools["stat"].tile([P, 2, 6], F32, tag="bnst")
    nc.vector.bn_stats(st[:, 0], x_t[:, 0:512])
    nc.vector.bn_stats(st[:, 1], x_t[:, 512:1024])
    mv = pools["stat"].tile([P, 2], F32, tag="bnmv")
    nc.vector.bn_aggr(mv, st)
    r = pools["stat"].tile([P, 1], F32, tag="bnr")
    nc.scalar.activation(r, mv[:, 1:2], AF.Sqrt, bias=pools["eps"], scale=1.0)
    nc.vector.reciprocal(r, r)
    nb = pools["stat"].tile([P, 1], F32, tag="bnnb")
    nc.vector.tensor_scalar(
        nb, mv[:, 0:1], scalar1=r, scalar2=-1.0, op0=OP.mult, op1=OP.mult
    )
    # xln = x * r - mu * r  (per-token scale/bias on the scalar engine)
    nc.scalar.activation(xln_t, x_t, AF.Identity, bias=nb, scale=r)
    if g_b is not None:
        nc.vector.tensor_tensor(xln_t, xln_t, g_b, OP.mult)
    if be_b is not None:
        nc.vector.tensor_tensor(xln_t, xln_t, be_b, OP.add)


def _transpose_into(nc, pools, src_t, dst, tt):
    """src_t [P, D] fp32 token-major tile tt -> dst [P, KO, T] f32r d-major.
    PSUM evacuation copies alternate DVE/ACT to balance engine load."""
    for ko in range(KO):
        psT = pools["psT"].tile([P, P], F32, tag="psT")
        nc.tensor.transpose(psT, src_t[:, ko * P : (ko + 1) * P], pools["ident"])
        out = dst[:, ko, tt * P : (tt + 1) * P]
        if ko % 2 == 0:
            nc.vector.tensor_copy(out, psT)
        else:
            nc.scalar.copy(out, psT)


def _bcast_tile(nc, pool, vec_ap, n, dtype=F32):
    """DMA-broadcast a [n] DRAM vector to a [P, n] SBUF tile."""
    t = pool.tile([P, n], dtype)
    src = bass.AP(
        tensor=vec_ap.tensor,
        offset=vec_ap.offset,
        ap=[[0, P]] + [list(x) for x in vec_ap.ap],
    )
    nc.sync.dma_start(t, src)
    return t


def build_program(flags):
    (use_bqkv, use_bo, use_bfc, use_bpr, use_g1, use_be1, use_g2, use_be2) = flags
    nc = bacc.Bacc(
        "TRN2", target_bir_lowering=False, debug=False, num_devices=N_CORES
    )
    x = nc.dram_tensor("x", [T, D], F32, kind="ExternalInput").ap()
    w_qkv = nc.dram_tensor("w_qkv", [D, 3 * D], F32R, kind="ExternalInput").ap()
    b_qkv = nc.dram_tensor("b_qkv", [3 * D], F32, kind="ExternalInput").ap()
    w_o = nc.dram_tensor("w_o", [D, D], F32R, kind="ExternalInput").ap()
    b_o = nc.dram_tensor("b_o", [D], F32, kind="ExternalInput").ap()
    g1 = nc.dram_tensor("g1", [D], F32, kind="ExternalInput").ap()
    be1 = nc.dram_tensor("be1", [D], F32, kind="ExternalInput").ap()
    w_fc = nc.dram_tensor("w_fc", [D, D4], F32R, kind="ExternalInput").ap()
    b_fc = nc.dram_tensor("b_fc", [D4], F32, kind="ExternalInput").ap()
    w_pr = nc.dram_tensor("w_pr", [D4, D], F32R, kind="ExternalInput").ap()
    b_pr = nc.dram_tensor("b_pr", [D], F32, kind="ExternalInput").ap()
    g2 = nc.dram_tensor("g2", [D], F32, kind="ExternalInput").ap()
    be2 = nc.dram_tensor("be2", [D], F32, kind="ExternalInput").ap()
    y = nc.dram_tensor("y", [T, D], F32, kind="ExternalOutput").ap()

    wqkv_ap = w_qkv.rearrange("(ko p) f -> p ko f", p=P)  # [128, 8, 3072]
    wo_ap = w_o.rearrange("(ko p) f -> p ko f", p=P)  # [128, 8, 1024]
    wfc_ap = w_fc.rearrange("(ko p) f -> p ko f", p=P)  # [128, 8, 4096]
    wpr_ap = w_pr.rearrange("(fo p) f -> p fo f", p=P)  # [128, 32, 1024]

    with tile.TileContext(nc) as tc, ExitStack() as ctx:
        consts = ctx.enter_context(tc.tile_pool(name="consts", bufs=1))
        resident = ctx.enter_context(tc.tile_pool(name="resident", bufs=1))
        dram = ctx.enter_context(tc.tile_pool(name="dram", bufs=1, space="DRAM"))

        ident = consts.tile([P, P], F32)
        make_identity(nc, ident)
        eps_t = consts.tile([P, 1], F32)
        nc.vector.memset(eps_t, EPS)
        ones_c = consts.tile([P, 1], F32)
        nc.vector.memset(ones_c, 1.0)

        g1_b = _bcast_tile(nc, consts, g1, D) if use_g1 else None
        be1_b = _bcast_tile(nc, consts, be1, D) if use_be1 else None
        g2_b = _bcast_tile(nc, consts, g2, D) if use_g2 else None
        be2_b = _bcast_tile(nc, consts, be2, D) if use_be2 else None
        bo_b = _bcast_tile(nc, consts, b_o, D) if use_bo else None
        bpr_b = _bcast_tile(nc, consts, b_pr, D) if use_bpr else None
        bv_b = _bcast_tile(nc, consts, b_qkv[2 * D : 3 * D], D) if use_bqkv else None
        if use_bqkv:
            # Q/K biases per-feature, feature-major: [128, 16] (fchunk majors)
            bqk_t = consts.tile([P, 16], F32)
            nc.sync.dma_start(bqk_t, b_qkv[0 : 2 * D].rearrange("(c p) -> p c", p=P))
        if use_bfc:
            bfc_t = consts.tile([P, KO4], F32)
            nc.sync.dma_start(bfc_t, b_fc.rearrange("(c p) -> p c", p=P))
            bfc_sc = consts.tile([P, KO4], F32)
            nc.vector.tensor_scalar_mul(bfc_sc, bfc_t, 1.702)

        xln1 = resident.tile([P, KO, T], F32R, tag="xln")
        o_dram = dram.tile([D, T], F32R)
        x2_dram = dram.tile([T, D], F32)

        with tc.tile_pool(name="vpool", bufs=1) as vpool:  # phases 1-2
            v_sb = vpool.tile([P, TT, H, DH + 1], F32R, tag="v")
            wv_t = vpool.tile([P, KO, D], F32R, tag="wv")

            # ------------- Phase 0: LN1 + transpose to d-major ---------------
            with (
                tc.tile_pool(name="ph0", bufs=3) as ph0,
                tc.tile_pool(name="stat", bufs=6) as stat,
                tc.tile_pool(name="psT", bufs=4, space="PSUM") as psT,
            ):
                pools = {"stat": stat, "psT": psT, "ident": ident, "eps": eps_t}
                for tt in range(TT):
                    x_t = ph0.tile([P, D], F32, tag="x")
                    nc.sync.dma_start(x_t, x[tt * P : (tt + 1) * P, :])
                    xln_t = ph0.tile([P, D], F32, tag="xln_t")
                    _ln(nc, pools, x_t, xln_t, g1_b, be1_b)
                    _transpose_into(nc, pools, xln_t, xln1, tt)

            # ------------- Phase 1: V projection (token-major, augmented) ----
            nc.sync.dma_start(wv_t, wqkv_ap[:, :, 2 * D : 3 * D])
            nc.vector.tensor_copy(
                v_sb[:, :, :, DH], ones_c[:, 0:1, None].to_broadcast([P, TT, H])
            )
            with (
                tc.tile_pool(name="ph1", bufs=2) as ph1,
                tc.tile_pool(name="psV", bufs=2, space="PSUM") as psV,
            ):
                for tt in range(TT):
                    for half in range(2):
                        ps = psV.tile([P, 512], F32, tag="psV")
                        for ko in range(KO):
                            nc.tensor.matmul(
                                ps,
                                xln1[:, ko, tt * P : (tt + 1) * P],
                                wv_t[:, ko, half * 512 : (half + 1) * 512],
                                start=(ko == 0),
                                stop=(ko == KO - 1),
                            )
                        dst = v_sb[:, tt, 8 * half : 8 * (half + 1), 0:DH]
                        src = ps.rearrange("p (h dh) -> p h dh", dh=DH)
                        if use_bqkv:
                            tmp = ph1.tile([P, 512], F32, tag="vtmp")
                            nc.vector.tensor_tensor(
                                tmp, ps, bv_b[:, half * 512 : (half + 1) * 512],
                                OP.add,
                            )
                            nc.vector.tensor_copy(
                                dst, tmp.rearrange("p (h dh) -> p h dh", dh=DH)
                            )
                        else:
                            nc.vector.tensor_copy(dst, src)

            # ------------- Phase 2: per-head-pair QKV + attention ------------
            with (
                tc.tile_pool(name="qk", bufs=2) as qkpool,
                tc.tile_pool(name="wqk", bufs=4) as wqkpool,
                tc.tile_pool(name="att", bufs=3) as attpool,
                tc.tile_pool(name="den", bufs=3) as denpool,
                tc.tile_pool(name="psQK", bufs=2, space="PSUM") as psQK,
                tc.tile_pool(name="psS", bufs=2, space="PSUM") as psS,
                tc.tile_pool(name="psO", bufs=2, space="PSUM") as psO,
            ):
                for j in range(H // 2):  # head pairs (2j, 2j+1)
                    q_sb = qkpool.tile([P, T], F32R, tag="q")
                    k_sb = qkpool.tile([P, T], F32R, tag="k")
                    for which, dst_sb in ((0, q_sb), (1, k_sb)):
                        f0 = which * D + j * P
                        w_t = wqkpool.tile([P, KO, P], F32R, tag="wqk")
                        nc.sync.dma_start(w_t, wqkv_ap[:, :, f0 : f0 + P])
                        for qc in range(3):
                            ps = psQK.tile([P, 512], F32, tag="psQK")
                            for ko in range(KO):
                                nc.tensor.matmul(
                                    ps,
                                    w_t[:, ko],
                                    xln1[:, ko, qc * 512 : (qc + 1) * 512],
                                    start=(ko == 0),
                                    stop=(ko == KO - 1),
                                )
                            dst = dst_sb[:, qc * 512 : (qc + 1) * 512]
                            if use_bqkv:
                                nc.vector.tensor_scalar(
                                    dst, ps,
                                    scalar1=bqk_t[
                                        :, 8 * which + j : 8 * which + j + 1
                                    ],
                                    scalar2=None, op0=OP.add,
                                )
                            else:
                                nc.vector.tensor_copy(dst, ps)

                    for t0, n in SEQS:
                        nkt = n // P
                        for q0 in range(t0, t0 + n, 512):
                            qn = min(512, t0 + n - q0)
                            po = [
                                psO.tile(
                                    [DH + 1, 512], F32, tag="psO", name="po"
                                )[:, :qn]
                                for _ in range(2)
                            ]
                            for ki in range(nkt):
                                k0 = t0 + ki * P
                                # both heads' scores into one 2-bank psum tile
                                ps_s = psS.tile(
                                    [P, 2, 512], F32, tag="psS", name="ps_s"
                                )
                                for hl in range(2):
                                    nc.tensor.matmul(
                                        ps_s[:, hl, :qn],
                                        k_sb[64 * hl : 64 * (hl + 1), k0 : k0 + P],
                                        q_sb[
                                            64 * hl : 64 * (hl + 1), q0 : q0 + qn
                                        ],
                                        tile_position=(64 * hl, 0),
                                    )
                                # one fused exp for both heads
                                e_t = attpool.tile(
                                    [P, 2, 512], F32R, tag="e", name="e_t"
                                )
                                nc.scalar.activation(
                                    e_t[:, :, :qn],
                                    ps_s[:, :, :qn],
                                    AF.Exp,
                                    scale=1.0 / np.sqrt(DH),
                                )
                                for hl in range(2):
                                    nc.tensor.matmul(
                                        po[hl],
                                        v_sb[:, k0 // P, 2 * j + hl, :],
                                        e_t[:, hl, :qn],
                                        start=(ki == 0),
                                        stop=(ki == nkt - 1),
                                    )
                            for hl in range(2):
                                h = 2 * j + hl
                                rden = denpool.tile(
                                    [1, 512], F32, tag="rden", name="rden"
                                )[:, :qn]
                                nc.vector.reciprocal(rden, po[hl][DH : DH + 1, :])
                                rb = denpool.tile(
                                    [DH, 512], F32, tag="rb", name="rb"
                                )[:, :qn]
                                nc.gpsimd.partition_broadcast(rb, rden)
                                o_t = attpool.tile(
                                    [DH, 512], F32R, tag="ot", name="o_t"
                                )[:, :qn]
                                nc.vector.tensor_tensor(
                                    o_t, po[hl][0:DH, :], rb, OP.mult
                                )
                                nc.sync.dma_start(
                                    o_dram[h * DH : (h + 1) * DH, q0 : q0 + qn],
                                    o_t,
                                )

        # ---------------- Phase 3: out-proj + residual + LN2 + transpose -----
        xln2 = resident.tile([P, KO, T], F32R, tag="xln")
        with (
            tc.tile_pool(name="ph3", bufs=4) as ph3,
            tc.tile_pool(name="ph3w", bufs=1) as ph3w,
            tc.tile_pool(name="ord", bufs=6) as ordpool,
            tc.tile_pool(name="stat3", bufs=6) as stat3,
            tc.tile_pool(name="psM", bufs=4, space="PSUM") as psM,
            tc.tile_pool(name="psT3", bufs=3, space="PSUM") as psT3,
        ):
            pools3 = {"stat": stat3, "psT": psT3, "ident": ident, "eps": eps_t}
            wo_t = ph3w.tile([P, KO, D], F32R, tag="wo")
            nc.sync.dma_start(wo_t, wo_ap)
            for tt in range(TT):
                x_t = ph3.tile([P, D], F32, tag="xr")
                nc.sync.dma_start(x_t, x[tt * P : (tt + 1) * P, :])
                ps2 = [
                    psM.tile([P, 512], F32, tag="psM", name=f"ps2_{_h}")
                    for _h in range(2)
                ]
                for ko in range(KO):
                    o_t = ordpool.tile([P, P], F32R, tag="ord")
                    nc.sync.dma_start(
                        o_t, o_dram[ko * P : (ko + 1) * P, tt * P : (tt + 1) * P]
                    )
                    for half in range(2):
                        nc.tensor.matmul(
                            ps2[half],
                            o_t,
                            wo_t[:, ko, half * 512 : (half + 1) * 512],
                            start=(ko == 0),
                            stop=(ko == KO - 1),
                        )
                x2_t = ph3.tile([P, D], F32, tag="x2")
                for half in range(2):
                    sl = slice(half * 512, (half + 1) * 512)
                    nc.vector.tensor_tensor(x2_t[:, sl], ps2[half], x_t[:, sl], OP.add)
                if use_bo:
                    nc.vector.tensor_tensor(x2_t, x2_t, bo_b, OP.add)
                nc.sync.dma_start(x2_dram[tt * P : (tt + 1) * P, :], x2_t)
                xln2_t = ph3.tile([P, D], F32, tag="xln2_t")
                _ln(nc, pools3, x2_t, xln2_t, g2_b, be2_b)
                _transpose_into(nc, pools3, xln2_t, xln2, tt)

        # ---------------- Phase 4: MLP (QuickGELU), y = x2 + mlp -------------
        # Token groups of TG=768; u kept on-chip per group; PSUM: 2 banks for
        # FC (N=512/256 sub-chunks) + 6 banks for the 6x128-token y strips.
        with (
            tc.tile_pool(name="ph4u", bufs=1) as ph4u,
            tc.tile_pool(name="wfc", bufs=3) as wfcpool,
            tc.tile_pool(name="wpr", bufs=3) as wprpool,
            tc.tile_pool(name="ph4t", bufs=3) as ph4t,
            tc.tile_pool(name="ph4x", bufs=12) as ph4x,
            tc.tile_pool(name="ph4e", bufs=3) as ph4e,
            tc.tile_pool(name="psU", bufs=2, space="PSUM") as psU,
            tc.tile_pool(name="psY", bufs=6, space="PSUM") as psY,
        ):
            NSUB = ((0, 512), (512, 256))  # FC sub-chunks within the group
            for tg in range(NTG):
                u_tg = ph4u.tile([P, KO4, TG], F32R, tag="u")
                for fc in range(KO4):
                    wfc_t = wfcpool.tile([P, KO, P], F32R, tag="wfc")
                    nc.sync.dma_start(wfc_t, wfc_ap[:, :, fc * P : (fc + 1) * P])
                    for s0, sn in NSUB:
                        psu = psU.tile([P, 512], F32, tag="psU", name="psu")[:, :sn]
                        for ko in range(KO):
                            nc.tensor.matmul(
                                psu,
                                wfc_t[:, ko],
                                xln2[:, ko, tg * TG + s0 : tg * TG + s0 + sn],
                                start=(ko == 0),
                                stop=(ko == KO - 1),
                            )
                        sig = ph4t.tile([P, 512], F32, tag="sig", name="sig")[:, :sn]
                        usl = u_tg[:, fc, s0 : s0 + sn]
                        if use_bfc:
                            nc.scalar.activation(
                                sig, psu, AF.Sigmoid, scale=1.702,
                                bias=bfc_sc[:, fc : fc + 1],
                            )
                            ub = ph4t.tile([P, 512], F32, tag="ub", name="ub")[:, :sn]
                            nc.vector.tensor_scalar(
                                ub, psu, scalar1=bfc_t[:, fc : fc + 1],
                                scalar2=None, op0=OP.add,
                            )
                            nc.vector.tensor_tensor(usl, ub, sig, OP.mult)
                        else:
                            nc.scalar.activation(sig, psu, AF.Sigmoid, scale=1.702)
                            nc.vector.tensor_tensor(usl, psu, sig, OP.mult)
                for dhalf in range(2):
                    dsl = slice(dhalf * 512, (dhalf + 1) * 512)
                    psy = [
                        psY.tile([P, 512], F32, tag="psY", name=f"psy_{_t}")
                        for _t in range(TG // P)
                    ]
                    x2_ts = []
                    for tc_ in range(TG // P):
                        rows = slice(tg * TG + tc_ * P, tg * TG + (tc_ + 1) * P)
                        x2_t = ph4x.tile([P, 512], F32, tag="x2r", name="x2_t")
                        nc.sync.dma_start(x2_t, x2_dram[rows, dsl])
                        x2_ts.append(x2_t)
                    for fc in range(KO4):
                        wpr_t = wprpool.tile([P, 512], F32R, tag="wpr")
                        nc.sync.dma_start(wpr_t, wpr_ap[:, fc, dsl])
                        for tc_ in range(TG // P):
                            nc.tensor.matmul(
                                psy[tc_],
                                u_tg[:, fc, tc_ * P : (tc_ + 1) * P],
                                wpr_t,
                                start=(fc == 0),
                                stop=(fc == KO4 - 1),
                            )
                    for tc_ in range(TG // P):
                        rows = slice(tg * TG + tc_ * P, tg * TG + (tc_ + 1) * P)
                        yv = ph4e.tile([P, 512], F32, tag="yv")
                        nc.vector.tensor_tensor(yv, psy[tc_], x2_ts[tc_], OP.add)
                        if use_bpr:
                            nc.vector.tensor_tensor(yv, yv, bpr_b[:, dsl], OP.add)
                        nc.sync.dma_start(y[rows, dsl], yv)

    nc.compile()
    return nc


def _get_program(flags):
    if flags not in _cache:
        _cache[flags] = build_program(flags)
    return _cache[flags]


def _run(x1, x2, w_qkv, b_qkv, w_o, b_o, g1, be1, w_fc, b_fc, w_pr, b_pr, g2, be2,
         **run_kwargs):
    x1 = np.ascontiguousarray(np.asarray(x1, dtype=np.float32))
    x2 = np.ascontiguousarray(np.asarray(x2, dtype=np.float32))
    arrs = dict(
        w_qkv=w_qkv, b_qkv=b_qkv, w_o=w_o, b_o=b_o, g1=g1, be1=be1,
        w_fc=w_fc, b_fc=b_fc, w_pr=w_pr, b_pr=b_pr, g2=g2, be2=be2,
    )
    arrs = {
        k: np.ascontiguousarray(np.asarray(v, dtype=np.float32))
        for k, v in arrs.items()
    }
    flags = (
        bool(np.any(arrs["b_qkv"])),
        bool(np.any(arrs["b_o"])),
        bool(np.any(arrs["b_fc"])),
        bool(np.any(arrs["b_pr"])),
        bool(np.any(arrs["g1"] != 1.0)),
        bool(np.any(arrs["be1"])),
        bool(np.any(arrs["g2"] != 1.0)),
        bool(np.any(arrs["be2"])),
    )
    nc = _get_program(flags)

    in_maps = []
    for i in range(N_CORES):
        shard = np.concatenate([x1[i], x2[2 * i], x2[2 * i + 1]], axis=0)
        m = {"x": np.ascontiguousarray(shard)}
        m.update(arrs)
        in_maps.append(m)

    res = run_bass_kernel_spmd(
        nc, in_maps, core_ids=list(range(N_CORES)), **run_kwargs
    )
    y1 = np.empty((8, 1024, D), dtype=np.float32)
    y2 = np.empty((16, 256, D), dtype=np.float32)
    for i in range(N_CORES):
        yi = res.results[i]["y"]
        y1[i] = yi[0:1024]
        y2[2 * i] = yi[1024:1280]
        y2[2 * i + 1] = yi[1280:1536]
    return y1, y2, res


def kernel(x1, x2, w_qkv, b_qkv, w_o, b_o, g1, be1, w_fc, b_fc, w_pr, b_pr, g2, be2):
    y1, y2, _ = _run(
        x1, x2, w_qkv, b_qkv, w_o, b_o, g1, be1,
        w_fc, b_fc, w_pr, b_pr, g2, be2,
    )
    return (y1, y2)
